# revision 39
# baseline (speedup 1.0000x reference)
"""Trainium2 Bass kernel for dilated (LongNet-style) sparse attention.

Model: B=1, T=2048, D=1024, H=16 heads (dh=64), RoPE over full model dim,
per-head multi-scale dilated mask with count-multiplicity, causal softmax,
output projection.

Sharding: head-parallel — core c (0..7) computes heads {c, c+8}. The head
pair is chosen so rotate_half's column partners (j <-> j+512) stay inside
the core's 128 projection columns. Each core produces a partial output
(its heads' contribution through the column-slice of Wo); the host sums
the 8 partials (column-parallel linear => reduce epilogue).

Sparsity: for head h, any valid (i, j) with j outside i's 128-block has
j === h (mod 64). So attention per (head, 128-row-block) is dense over
[own 128 block] ++ [32 strided extra columns], i.e. 160 columns, with an
exact mask (cmul = count multiplicity, cadd = -1e30 invalid) built on the
host. Extra-column K/V are recomputed on device from host-sliced x rows
(xeT input) so the program stays SPMD-static (no per-core addressing).

All per-core data (weight slices, RoPE tables, masks) arrives as inputs.
"""

import math

import numpy as np
import ml_dtypes

import concourse.bass as bass
import concourse.bacc as bacc
import concourse.tile as tile
from concourse import mybir
from concourse.masks import make_identity
from concourse.bass_utils import run_bass_kernel_spmd

F32 = mybir.dt.float32
F32R = mybir.dt.float32r
BF16 = mybir.dt.bfloat16
F16 = mybir.dt.float16
AF = mybir.ActivationFunctionType
ALU = mybir.AluOpType
AX = mybir.AxisListType

H, T, D = 16, 2048, 1024
DH = D // H            # 64
NB = T // 128          # 16 row blocks
NG = NB // 2           # 8 groups of 2 blocks
NE = 32                # extra (strided) columns per head
TW = 128 + NE          # 160: attention tile width
GW = 2 * TW            # 320: group width (2 tiles)
NCORES = 8
BASE = 15000.0
NEG = -60000.0


# --------------------------------------------------------------------------
# device program
# --------------------------------------------------------------------------

def _emit(nc, tc, ctx, io):
    xTin, xeT, wqTd, wkTd, wvTd, woTd, cosT, sinT, ceT, seT, cmul, cadd, pout = io

    const = ctx.enter_context(tc.tile_pool(name="const", bufs=1))
    big = ctx.enter_context(tc.tile_pool(name="big", bufs=1))
    stg = ctx.enter_context(tc.tile_pool(name="stg", bufs=2))
    rope = ctx.enter_context(tc.tile_pool(name="rope", bufs=2))
    attn = ctx.enter_context(tc.tile_pool(name="attn", bufs=2))
    stage = ctx.enter_context(tc.tile_pool(name="stage", bufs=2))
    psA = ctx.enter_context(tc.tile_pool(name="psA", bufs=2, space="PSUM"))
    psSC = ctx.enter_context(tc.tile_pool(name="psSC", bufs=2, space="PSUM"))
    psPT = ctx.enter_context(tc.tile_pool(name="psPT", bufs=1, space="PSUM"))
    psOT = ctx.enter_context(tc.tile_pool(name="psOT", bufs=2, space="PSUM"))

    ident = const.tile([128, 128], F32)
    make_identity(nc, ident[:])
    identB = const.tile([128, 128], F16)
    make_identity(nc, identB[:])

    # alternate evacuation engine to balance DVE/ACT
    def evac(i, dst, src):
        if i % 2 == 0:
            nc.vector.tensor_copy(dst, src)
        else:
            nc.scalar.copy(dst, src)


    # ---- stage B: load fp16 weights directly (host supplies device layout) ----
    wqT = const.tile([128, D], F16)  # [d(chunked on part), c]
    wkT = const.tile([128, D], F16)
    wvT = const.tile([128, D], F16)
    woT = const.tile([128, D], F16)  # [c, u]
    nc.sync.dma_start(wkT[:], wkTd)
    nc.sync.dma_start(wvT[:], wvTd)
    nc.sync.dma_start(wqT[:], wqTd)
    nc.sync.dma_start(woT[:], woTd)
    xeSr = const.tile([128, 8 * 2 * NE], F16)
    nc.sync.dma_start(xeSr[:], xeT)
    ceS = const.tile([128, 2 * NE], F16)
    nc.sync.dma_start(ceS[:], ceT)
    seS = const.tile([128, 2 * NE], F16)
    nc.sync.dma_start(seS[:], seT)

    # ---- stage A: load x.T (host-transposed) and cast to f32r ----
    # t-slice-major (n outer) so projections for t-slice n can start and
    # finish while later slices stream in. cos/sin stream alongside.
    cosS = const.tile([128, T], F16)
    sinS = const.tile([128, T], F16)
    nc.sync.dma_start(cosS[:], cosT)
    nc.sync.dma_start(sinS[:], sinT)
    xT = big.tile([128, 8 * T], F16)  # d-chunk dc occupies cols [dc*T, dc*T+T)
    for np2 in range(2):
        for dc in range(8):
            nc.sync.dma_start(
                xT[:, dc * T + np2 * 1024: dc * T + np2 * 1024 + 1024],
                xTin[dc * 128:(dc + 1) * 128, np2 * 1024:(np2 + 1) * 1024])

    # ---- extra-row K/V (keRot [c, je], ve [je, c]) ----
    kvPs = psA.tile([128, 512], F32, tag="psA")
    kePs = kvPs[:, 0:64]
    vePs = kvPs[:, 64:128]
    for dc in range(8):
        nc.tensor.matmul(kePs, wkT[:, dc * 128:(dc + 1) * 128],
                         xeSr[:, dc * 64:(dc + 1) * 64],
                         start=(dc == 0), stop=(dc == 7))
    for dc in range(8):
        nc.tensor.matmul(vePs, wvT[:, dc * 128:(dc + 1) * 128],
                         xeSr[:, dc * 64:(dc + 1) * 64],
                         start=(dc == 0), stop=(dc == 7))
    tmpE = rope.tile([128, 2 * NE], F32, tag="tmpE")
    nc.scalar.copy(tmpE[0:64, :], kePs[64:128, :])
    nc.scalar.copy(tmpE[64:128, :], kePs[0:64, :])
    wE = rope.tile([128, 2 * NE], F32, tag="wE")
    nc.vector.tensor_mul(wE[:], kePs, ceS[:])
    nc.vector.tensor_mul(tmpE[:], tmpE[:], seS[:])
    keRot = const.tile([128, 2 * NE], F16)
    nc.vector.tensor_add(keRot[:], wE[:], tmpE[:])

    veT_sb = rope.tile([128, 2 * NE], F16, tag="veT")
    nc.vector.tensor_copy(veT_sb[:], vePs)
    ps2 = psPT.tile([128, 512], F16, tag="pt")
    nc.tensor.transpose(ps2[0:64, 0:128], veT_sb[:], identB[:])
    # [je, c] with each head's 32 rows re-based to partition 0:
    # cols [0:128) = head slot 0 rows, cols [128:256) = head slot 1 rows
    ve_sb = const.tile([32, 256], F16)
    nc.vector.tensor_copy(ve_sb[:, 0:128], ps2[0:32, 0:128])
    nc.vector.tensor_copy(ve_sb[:, 128:256], ps2[32:64, 0:128])

    # ---- stage C: projections + RoPE (slice-paced: n outer) ----
    qrot = big.tile([128, T], F16)
    krot = big.tile([128, T], F16)
    v_sb = big.tile([128, NB * 128], F16)  # [t(part), (tb, c)]
    for n in range(4):
        for ti, wT in enumerate([wqT, wkT, wvT]):
            ps = psA.tile([128, 512], F32, tag="psA")
            for dc in range(8):
                nc.tensor.matmul(
                    ps[:],
                    wT[:, dc * 128:(dc + 1) * 128],
                    xT[:, dc * T + n * 512: dc * T + n * 512 + 512],
                    start=(dc == 0), stop=(dc == 7))
            if ti < 2:
                dstRot = qrot if ti == 0 else krot
                w_ = rope.tile([128, 512], F32, tag="w_")
                nc.vector.tensor_mul(w_[:], ps[:], cosS[:, n * 512:(n + 1) * 512])
                u_ = rope.tile([128, 512], F32, tag="u_")
                nc.vector.tensor_mul(u_[0:64, :], ps[64:128, :],
                                     sinS[0:64, n * 512:(n + 1) * 512])
                nc.vector.tensor_mul(u_[64:128, :], ps[0:64, :],
                                     sinS[64:128, n * 512:(n + 1) * 512])
                nc.vector.tensor_add(dstRot[:, n * 512:(n + 1) * 512],
                                     w_[:], u_[:])
            else:
                vch = rope.tile([128, 512], F16, tag="vch")
                nc.scalar.copy(vch[:], ps[:])
                ps3 = psPT.tile([128, 512], F16, tag="pt")
                for i in range(4):
                    nc.tensor.transpose(ps3[:, i * 128:(i + 1) * 128],
                                        vch[:, i * 128:(i + 1) * 128], identB[:])
                evac(n, v_sb[:, n * 512:(n + 1) * 512], ps3[:])

    # ---- stage D: block-sparse attention (groups of 2 row-blocks) ----
    outT = big.tile([128, T], F16)  # [c(2 heads), t]
    for bg in range(NG):
        for s in range(2):
            cm = attn.tile([128, GW], F16, tag="cm")
            nc.scalar.dma_start(
                cm[:], cmul[(s * NG + bg) * 128:(s * NG + bg + 1) * 128, :])
            ca = attn.tile([128, GW], F16, tag="ca")
            nc.scalar.dma_start(
                ca[:], cadd[(s * NG + bg) * 128:(s * NG + bg + 1) * 128, :])

            sc = psSC.tile([128, 512], F32, tag="sc")
            for i in range(2):
                b = 2 * bg + i
                qs = qrot[64 * s:64 * s + 64, b * 128:(b + 1) * 128]
                nc.tensor.matmul(sc[:, i * TW:i * TW + 128], qs,
                                 krot[64 * s:64 * s + 64, b * 128:(b + 1) * 128],
                                 start=True, stop=True)
                nc.tensor.matmul(sc[:, i * TW + 128:i * TW + TW], qs,
                                 keRot[64 * s:64 * s + 64, NE * s:NE * s + NE],
                                 start=True, stop=True)

            att = attn.tile([128, GW], F32, tag="att")
            nc.vector.tensor_mul(att[:], sc[:, 0:GW], cm[:])
            nc.gpsimd.tensor_add(att[:], att[:], ca[:])
            att3 = att[:].rearrange("p (t y) -> p t y", y=TW)
            nmax = attn.tile([128, 2], F32, tag="nmax")
            nc.vector.tensor_reduce(nmax[:], att3, axis=AX.X, op=ALU.max,
                                    negate=True)
            rsum = attn.tile([128, 2], F32, tag="rsum")
            for i in range(2):
                nc.scalar.activation(att[:, i * TW:(i + 1) * TW],
                                     att[:, i * TW:(i + 1) * TW], AF.Exp,
                                     bias=nmax[:, i:i + 1],
                                     accum_out=rsum[:, i:i + 1])
            rinv = attn.tile([128, 2], F32, tag="rinv")
            nc.vector.reciprocal(rinv[:], rsum[:])
            pbf = attn.tile([128, GW], F16, tag="pbf")
            for i in range(2):
                nc.vector.tensor_scalar_mul(pbf[:, i * TW:(i + 1) * TW],
                                            att[:, i * TW:(i + 1) * TW],
                                            rinv[:, i:i + 1])

            pt = psPT.tile([128, 512], F16, tag="pt")
            ptx = psPT.tile([32, 256], F16, tag="ptx")
            for i in range(2):
                nc.tensor.transpose(pt[:, i * 128:(i + 1) * 128],
                                    pbf[:, i * TW:i * TW + 128], identB[:])
                nc.tensor.transpose(ptx[:, i * 128:(i + 1) * 128],
                                    pbf[:, i * TW + 128:i * TW + TW], identB[:])
            ptS = attn.tile([128, 256], F16, tag="ptS")
            nc.scalar.copy(ptS[:], pt[:, 0:256])
            ptxS = attn.tile([32, 256], F16, tag="ptxS", bufs=1)
            nc.scalar.copy(ptxS[:], ptx[:])

            ot = psOT.tile([64, 256], F32, tag="ot")
            for i in range(2):
                b = 2 * bg + i
                nc.tensor.matmul(ot[:, i * 128:(i + 1) * 128],
                                 v_sb[:, b * 128 + 64 * s: b * 128 + 64 * s + 64],
                                 ptS[:, i * 128:(i + 1) * 128],
                                 start=True, stop=False)
                nc.tensor.matmul(ot[:, i * 128:(i + 1) * 128],
                                 ve_sb[:, 128 * s + 64 * s:128 * s + 64 * s + 64],
                                 ptxS[:, i * 128:(i + 1) * 128],
                                 start=False, stop=True)
            nc.scalar.copy(outT[64 * s:64 * s + 64, bg * 256:(bg + 1) * 256],
                           ot[:])

    # ---- stage E: output projection (partial over this core's columns) ----
    for tb in range(NB):
        st = stage.tile([128, D], F16, tag="st")
        for uc in range(2):
            pr = psA.tile([128, 512], F32, tag="psA")
            nc.tensor.matmul(pr[:],
                             outT[:, tb * 128:(tb + 1) * 128],
                             woT[:, uc * 512:(uc + 1) * 512],
                             start=True, stop=True)
            evac(uc, st[:, uc * 512:(uc + 1) * 512], pr[:])
        nc.scalar.dma_start(pout[tb * 128:(tb + 1) * 128, :], st[:])


def build_program():
    nc = bacc.Bacc("TRN2", target_bir_lowering=False, debug=False)
    io = (
        nc.dram_tensor("xTin", [D, T], F16, kind="ExternalInput").ap(),
        nc.dram_tensor("xeT", [128, 8 * 2 * NE], F16, kind="ExternalInput").ap(),
        nc.dram_tensor("wqT", [128, D], F16, kind="ExternalInput").ap(),
        nc.dram_tensor("wkT", [128, D], F16, kind="ExternalInput").ap(),
        nc.dram_tensor("wvT", [128, D], F16, kind="ExternalInput").ap(),
        nc.dram_tensor("woT", [128, D], F16, kind="ExternalInput").ap(),
        nc.dram_tensor("cosT", [128, T], F16, kind="ExternalInput").ap(),
        nc.dram_tensor("sinT", [128, T], F16, kind="ExternalInput").ap(),
        nc.dram_tensor("ceT", [128, 2 * NE], F16, kind="ExternalInput").ap(),
        nc.dram_tensor("seT", [128, 2 * NE], F16, kind="ExternalInput").ap(),
        nc.dram_tensor("cmul", [2 * NG * 128, GW], F16, kind="ExternalInput").ap(),
        nc.dram_tensor("cadd", [2 * NG * 128, GW], F16, kind="ExternalInput").ap(),
        nc.dram_tensor("partial", [T, D], F16, kind="ExternalOutput").ap(),
    )
    from contextlib import ExitStack
    with tile.TileContext(nc) as tc, ExitStack() as ctx:
        _emit(nc, tc, ctx, io)
    nc.compile()
    return nc


# --------------------------------------------------------------------------
# host-side input preparation
# --------------------------------------------------------------------------

def _count_mask():
    """count[h, i, j] (uint8) = multiplicity of dilation-scale coverage."""
    idx = np.arange(T)
    count = np.zeros((H, T, T), np.uint8)
    for X in range(int(math.log2(T))):
        S, r = 2 ** (X + 2), 2 ** X
        same_seg = (idx[:, None] // S) == (idx[None, :] // S)
        for h in range(H):
            ok = (idx % r) == (h % r)
            count[h] += (same_seg & ok[:, None] & ok[None, :]).astype(np.uint8)
    return count


def _extra_rows(h):
    return np.concatenate([h + 128 * np.arange(16), h + 64 + 128 * np.arange(16)])


_STATIC = None


def _static_inputs():
    """Everything that doesn't depend on runtime tensor values."""
    global _STATIC
    if _STATIC is not None:
        return _STATIC
    count = _count_mask()
    # match the reference's fp32 angle arithmetic exactly (t*f rounds in fp32)
    inv_freq = (1.0 / (np.float32(BASE)
                       ** (np.arange(0, D, 2, dtype=np.float32) / np.float32(D)))
                ).astype(np.float32)
    tpos = np.arange(T, dtype=np.float32)

    per_core = []
    ii = np.arange(T)
    for c in range(NCORES):
        heads = [c, c + 8]
        wrows = np.concatenate([np.arange(64 * c, 64 * c + 64),
                                np.arange(512 + 64 * c, 512 + 64 * c + 64)])
        erows = np.concatenate([_extra_rows(h) for h in heads])  # 64
        # RoPE tables in transposed layout [c-dim row, t]
        f = inv_freq[wrows % 512]                                 # [128]
        ang = (tpos[None, :] * f[:, None]).astype(np.float64)     # fp32-rounded angle
        cosT = np.cos(ang).astype(np.float32)
        sign = np.where(np.arange(128) < 64, -1.0, 1.0)[:, None]
        sinT = (np.sin(ang) * sign).astype(np.float32)
        ceT = cosT[:, erows].copy()
        seT = sinT[:, erows].copy()
        # masks [2, NG, 128, GW]
        cmul = np.zeros((2, NG, 128, GW), np.float32)
        cadd = np.full((2, NG, 128, GW), NEG, np.float32)
        for s, h in enumerate(heads):
            ec = _extra_rows(h)
            for b in range(NB):
                rows = slice(128 * b, 128 * b + 128)
                iblk = ii[rows][:, None]
                co = (b % 2) * TW
                jblk = np.arange(128 * b, 128 * b + 128)[None, :]
                cnt = count[h, rows, 128 * b:128 * b + 128].astype(np.float32)
                val = (cnt > 0) & (jblk <= iblk)
                cmul[s, b // 2, :, co:co + 128] = np.where(val, cnt, 0.0)
                cadd[s, b // 2, :, co:co + 128] = np.where(val, 0.0, NEG)
                je = ec[None, :]
                cnt_e = count[h, rows, :][:, ec].astype(np.float32)
                inb = (je >= 128 * b) & (je < 128 * b + 128)
                val_e = (cnt_e > 0) & (je <= iblk) & ~inb
                cmul[s, b // 2, :, co + 128:co + TW] = np.where(val_e, cnt_e, 0.0)
                cadd[s, b // 2, :, co + 128:co + TW] = np.where(val_e, 0.0, NEG)
        per_core.append(dict(
            wrows=wrows, erows=erows,
            cosT=cosT.astype(np.float16), sinT=sinT.astype(np.float16),
            ceT=ceT.astype(np.float16), seT=seT.astype(np.float16),
            cmul=cmul.reshape(2 * NG * 128, GW).astype(np.float16),
            cadd=cadd.reshape(2 * NG * 128, GW).astype(np.float16),
        ))
    _STATIC = per_core
    return per_core


_NC = None


def _program():
    global _NC
    if _NC is None:
        _NC = build_program()
    return _NC


def build_in_maps(x, Wq, Wk, Wv, Wo):
    x2 = np.ascontiguousarray(x.reshape(T, D), np.float32)
    xT2h = np.ascontiguousarray(x2.T).astype(np.float16)
    static = _static_inputs()
    in_maps = []
    for c in range(NCORES):
        st = static[c]
        wr = st["wrows"]
        def dev_wT(w):
            # [128 c, D] slice -> device layout [128 p, (chunk, c)]
            return np.ascontiguousarray(
                w.T.reshape(8, 128, 128).transpose(1, 0, 2).reshape(128, 8 * 128))

        xe = x2[st["erows"], :].T  # [D, 64]
        in_maps.append({
            "xTin": xT2h,
            "xeT": np.ascontiguousarray(
                xe.reshape(8, 128, 2 * NE).transpose(1, 0, 2)
                  .reshape(128, -1)).astype(np.float16),
            "wqT": dev_wT(Wq[wr, :]).astype(np.float16),
            "wkT": dev_wT(Wk[wr, :]).astype(np.float16),
            "wvT": dev_wT(Wv[wr, :]).astype(np.float16),
            "woT": np.ascontiguousarray(Wo[:, wr].T).astype(np.float16),
            "cosT": st["cosT"],
            "sinT": st["sinT"],
            "ceT": st["ceT"],
            "seT": st["seT"],
            "cmul": st["cmul"],
            "cadd": st["cadd"],
        })
    return in_maps


def kernel(x, Wq, bq, Wk, bk, Wv, bv, Wo, bo, **_):
    x = np.asarray(x, np.float32)
    Wq, Wk, Wv, Wo = (np.asarray(a, np.float32) for a in (Wq, Wk, Wv, Wo))
    bq, bk, bv, bo = (np.asarray(a, np.float32) for a in (bq, bk, bv, bo))
    assert not bq.any() and not bk.any(), "nonzero q/k biases unsupported"

    nc = _program()
    in_maps = build_in_maps(x, Wq, Wk, Wv, Wo)
    res = run_bass_kernel_spmd(nc, in_maps, list(range(NCORES)))
    out = np.zeros((T, D), np.float32)
    for r in res.results:
        out += r["partial"].astype(np.float32)
    # v/o biases are exact host-side epilogues: sum_j p_ij = 1 per row
    out += (Wo @ bv + bo)[None, :]
    return out.reshape(1, T, D)


# revision 40
# speedup vs baseline: 1.1813x; 1.1813x over previous
"""Trainium2 Bass kernel for dilated (LongNet-style) sparse attention.

Model: B=1, T=2048, D=1024, H=16 heads (dh=64), RoPE over full model dim,
per-head multi-scale dilated mask with count-multiplicity, causal softmax,
output projection.

Sharding: head-parallel — core c (0..7) computes heads {c, c+8}. The head
pair is chosen so rotate_half's column partners (j <-> j+512) stay inside
the core's 128 projection columns. Each core produces a partial output
(its heads' contribution through the column-slice of Wo); the host sums
the 8 partials (column-parallel linear => reduce epilogue).

Sparsity: for head h, any valid (i, j) with j outside i's 128-block has
j === h (mod 64). So attention per (head, 128-row-block) is dense over
[own 128 block] ++ [32 strided extra columns], i.e. 160 columns, with an
exact mask (cmul = count multiplicity, cadd = -1e30 invalid) built on the
host. Extra-column K/V are recomputed on device from host-sliced x rows
(xeT input) so the program stays SPMD-static (no per-core addressing).

All per-core data (weight slices, RoPE tables, masks) arrives as inputs.
"""

import math

import numpy as np
import ml_dtypes

import concourse.bass as bass
import concourse.bacc as bacc
import concourse.tile as tile
from concourse import mybir
from concourse.masks import make_identity
from concourse.bass_utils import run_bass_kernel_spmd

F32 = mybir.dt.float32
F32R = mybir.dt.float32r
BF16 = mybir.dt.bfloat16
F16 = mybir.dt.float16
AF = mybir.ActivationFunctionType
ALU = mybir.AluOpType
AX = mybir.AxisListType

H, T, D = 16, 2048, 1024
DH = D // H            # 64
NB = T // 128          # 16 row blocks
NG = NB // 2           # 8 groups of 2 blocks
NE = 32                # extra (strided) columns per head
TW = 128 + NE          # 160: attention tile width
GW = 2 * TW            # 320: group width (2 tiles)
NCORES = 8
BASE = 15000.0
NEG = -60000.0


# --------------------------------------------------------------------------
# device program
# --------------------------------------------------------------------------

def _emit(nc, tc, ctx, io):
    xTin, xeT, wqTd, wkTd, wvTd, woTd, cosT, sinT, ceT, seT, cmul, cadd, pout = io

    const = ctx.enter_context(tc.tile_pool(name="const", bufs=1))
    big = ctx.enter_context(tc.tile_pool(name="big", bufs=1))
    stg = ctx.enter_context(tc.tile_pool(name="stg", bufs=2))
    rope = ctx.enter_context(tc.tile_pool(name="rope", bufs=2))
    attn = ctx.enter_context(tc.tile_pool(name="attn", bufs=2))
    stage = ctx.enter_context(tc.tile_pool(name="stage", bufs=2))
    psA = ctx.enter_context(tc.tile_pool(name="psA", bufs=2, space="PSUM"))
    psSC = ctx.enter_context(tc.tile_pool(name="psSC", bufs=2, space="PSUM"))
    psPT = ctx.enter_context(tc.tile_pool(name="psPT", bufs=1, space="PSUM"))
    psOT = ctx.enter_context(tc.tile_pool(name="psOT", bufs=2, space="PSUM"))

    ident = const.tile([128, 128], F32)
    make_identity(nc, ident[:])
    identB = const.tile([128, 128], F16)
    make_identity(nc, identB[:])

    # alternate evacuation engine to balance DVE/ACT
    def evac(i, dst, src):
        if i % 2 == 0:
            nc.vector.tensor_copy(dst, src)
        else:
            nc.scalar.copy(dst, src)


    # ---- stage B: load fp16 weights directly (host supplies device layout) ----
    wqT = const.tile([128, D], F16)  # [d(chunked on part), c]
    wkT = const.tile([128, D], F16)
    wvT = const.tile([128, D], F16)
    woT = const.tile([128, D], F16)  # [c, u]
    nc.sync.dma_start(wkT[:], wkTd)
    nc.sync.dma_start(wvT[:], wvTd)
    nc.sync.dma_start(wqT[:], wqTd)
    nc.sync.dma_start(woT[:], woTd)
    xeSr = const.tile([128, 8 * 2 * NE], F16)
    nc.sync.dma_start(xeSr[:], xeT)
    ceS = const.tile([128, 2 * NE], F16)
    nc.sync.dma_start(ceS[:], ceT)
    seS = const.tile([128, 2 * NE], F16)
    nc.sync.dma_start(seS[:], seT)

    # ---- stage A: load x.T (host-transposed) and cast to f32r ----
    # t-slice-major (n outer) so projections for t-slice n can start and
    # finish while later slices stream in. cos/sin stream alongside.
    cosS = const.tile([128, T], F16)
    sinS = const.tile([128, T], F16)
    nc.sync.dma_start(cosS[:], cosT)
    nc.sync.dma_start(sinS[:], sinT)
    xT = big.tile([128, 8 * T], F16)  # d-chunk dc occupies cols [dc*T, dc*T+T)
    for np2 in range(2):
        for dc in range(8):
            nc.sync.dma_start(
                xT[:, dc * T + np2 * 1024: dc * T + np2 * 1024 + 1024],
                xTin[dc * 128:(dc + 1) * 128, np2 * 1024:(np2 + 1) * 1024])

    # ---- extra-row K/V (keRot [c, je], ve [je, c]) ----
    kvPs = psA.tile([128, 512], F32, tag="psA")
    kePs = kvPs[:, 0:64]
    vePs = kvPs[:, 64:128]
    for dc in range(8):
        nc.tensor.matmul(kePs, wkT[:, dc * 128:(dc + 1) * 128],
                         xeSr[:, dc * 64:(dc + 1) * 64],
                         start=(dc == 0), stop=(dc == 7))
    for dc in range(8):
        nc.tensor.matmul(vePs, wvT[:, dc * 128:(dc + 1) * 128],
                         xeSr[:, dc * 64:(dc + 1) * 64],
                         start=(dc == 0), stop=(dc == 7))
    tmpE = rope.tile([128, 2 * NE], F32, tag="tmpE")
    nc.scalar.copy(tmpE[0:64, :], kePs[64:128, :])
    nc.scalar.copy(tmpE[64:128, :], kePs[0:64, :])
    wE = rope.tile([128, 2 * NE], F32, tag="wE")
    nc.vector.tensor_mul(wE[:], kePs, ceS[:])
    nc.vector.tensor_mul(tmpE[:], tmpE[:], seS[:])
    keRot = const.tile([128, 2 * NE], F16)
    nc.vector.tensor_add(keRot[:], wE[:], tmpE[:])

    veT_sb = rope.tile([128, 2 * NE], F16, tag="veT")
    nc.vector.tensor_copy(veT_sb[:], vePs)
    ps2 = psPT.tile([128, 512], F16, tag="pt")
    nc.tensor.transpose(ps2[0:64, 0:128], veT_sb[:], identB[:])
    # [je, c] with each head's 32 rows re-based to partition 0:
    # cols [0:128) = head slot 0 rows, cols [128:256) = head slot 1 rows
    ve_sb = const.tile([32, 256], F16)
    nc.vector.tensor_copy(ve_sb[:, 0:128], ps2[0:32, 0:128])
    nc.vector.tensor_copy(ve_sb[:, 128:256], ps2[32:64, 0:128])

    # ---- stage C: projections + RoPE (slice-paced: n outer) ----
    qrot = big.tile([128, T], F16)
    krot = big.tile([128, T], F16)
    v_sb = big.tile([128, NB * 128], F16)  # [t(part), (tb, c)]
    for n in range(4):
        for ti, wT in enumerate([wqT, wkT, wvT]):
            ps = psA.tile([128, 512], F32, tag="psA")
            for dc in range(8):
                nc.tensor.matmul(
                    ps[:],
                    wT[:, dc * 128:(dc + 1) * 128],
                    xT[:, dc * T + n * 512: dc * T + n * 512 + 512],
                    start=(dc == 0), stop=(dc == 7))
            if ti < 2:
                dstRot = qrot if ti == 0 else krot
                w_ = rope.tile([128, 512], F32, tag="w_")
                nc.vector.tensor_mul(w_[:], ps[:], cosS[:, n * 512:(n + 1) * 512])
                u_ = rope.tile([128, 512], F32, tag="u_")
                nc.vector.tensor_mul(u_[0:64, :], ps[64:128, :],
                                     sinS[0:64, n * 512:(n + 1) * 512])
                nc.vector.tensor_mul(u_[64:128, :], ps[0:64, :],
                                     sinS[64:128, n * 512:(n + 1) * 512])
                nc.vector.tensor_add(dstRot[:, n * 512:(n + 1) * 512],
                                     w_[:], u_[:])
            else:
                vch = rope.tile([128, 512], F16, tag="vch")
                nc.scalar.copy(vch[:], ps[:])
                ps3 = psPT.tile([128, 512], F16, tag="pt")
                for i in range(4):
                    nc.tensor.transpose(ps3[:, i * 128:(i + 1) * 128],
                                        vch[:, i * 128:(i + 1) * 128], identB[:])
                evac(n, v_sb[:, n * 512:(n + 1) * 512], ps3[:])

    # ---- stage D: block-sparse attention (groups of 2 row-blocks) ----
    outT = big.tile([128, T], F16)  # [c(2 heads), t]
    for bg in range(NG):
        for s in range(2):
            cm = attn.tile([128, GW], F16, tag="cm")
            nc.gpsimd.dma_start(
                cm[:], cmul[(s * NG + bg) * 128:(s * NG + bg + 1) * 128, :])
            ca = attn.tile([128, GW], F16, tag="ca")
            nc.gpsimd.dma_start(
                ca[:], cadd[(s * NG + bg) * 128:(s * NG + bg + 1) * 128, :])

            sc = psSC.tile([128, 512], F32, tag="sc")
            for i in range(2):
                b = 2 * bg + i
                qs = qrot[64 * s:64 * s + 64, b * 128:(b + 1) * 128]
                nc.tensor.matmul(sc[:, i * TW:i * TW + 128], qs,
                                 krot[64 * s:64 * s + 64, b * 128:(b + 1) * 128],
                                 start=True, stop=True)
                nc.tensor.matmul(sc[:, i * TW + 128:i * TW + TW], qs,
                                 keRot[64 * s:64 * s + 64, NE * s:NE * s + NE],
                                 start=True, stop=True)

            att = attn.tile([128, GW], F32, tag="att")
            nc.vector.tensor_mul(att[:], sc[:, 0:GW], cm[:])
            nc.gpsimd.tensor_add(att[:], att[:], ca[:])
            att3 = att[:].rearrange("p (t y) -> p t y", y=TW)
            nmax = attn.tile([128, 2], F32, tag="nmax")
            nc.vector.tensor_reduce(nmax[:], att3, axis=AX.X, op=ALU.max,
                                    negate=True)
            rsum = attn.tile([128, 2], F32, tag="rsum")
            for i in range(2):
                nc.scalar.activation(att[:, i * TW:(i + 1) * TW],
                                     att[:, i * TW:(i + 1) * TW], AF.Exp,
                                     bias=nmax[:, i:i + 1],
                                     accum_out=rsum[:, i:i + 1])
            rinv = attn.tile([128, 2], F32, tag="rinv")
            nc.vector.reciprocal(rinv[:], rsum[:])
            pbf = attn.tile([128, GW], F16, tag="pbf")
            for i in range(2):
                nc.vector.tensor_scalar_mul(pbf[:, i * TW:(i + 1) * TW],
                                            att[:, i * TW:(i + 1) * TW],
                                            rinv[:, i:i + 1])

            pt = psPT.tile([128, 512], F16, tag="pt")
            ptx = psPT.tile([32, 256], F16, tag="ptx")
            for i in range(2):
                nc.tensor.transpose(pt[:, i * 128:(i + 1) * 128],
                                    pbf[:, i * TW:i * TW + 128], identB[:])
                nc.tensor.transpose(ptx[:, i * 128:(i + 1) * 128],
                                    pbf[:, i * TW + 128:i * TW + TW], identB[:])
            ptS = attn.tile([128, 256], F16, tag="ptS")
            nc.vector.tensor_copy(ptS[:], pt[:, 0:256])
            ptxS = attn.tile([32, 256], F16, tag="ptxS", bufs=1)
            nc.vector.tensor_copy(ptxS[:], ptx[:])

            ot = psOT.tile([64, 256], F32, tag="ot")
            for i in range(2):
                b = 2 * bg + i
                nc.tensor.matmul(ot[:, i * 128:(i + 1) * 128],
                                 v_sb[:, b * 128 + 64 * s: b * 128 + 64 * s + 64],
                                 ptS[:, i * 128:(i + 1) * 128],
                                 start=True, stop=False)
                nc.tensor.matmul(ot[:, i * 128:(i + 1) * 128],
                                 ve_sb[:, 128 * s + 64 * s:128 * s + 64 * s + 64],
                                 ptxS[:, i * 128:(i + 1) * 128],
                                 start=False, stop=True)
            nc.scalar.copy(outT[64 * s:64 * s + 64, bg * 256:(bg + 1) * 256],
                           ot[:])

    # ---- stage E: output projection (partial over this core's columns) ----
    for tb in range(NB):
        st = stage.tile([128, D], F16, tag="st")
        for uc in range(2):
            pr = psA.tile([128, 512], F32, tag="psA")
            nc.tensor.matmul(pr[:],
                             outT[:, tb * 128:(tb + 1) * 128],
                             woT[:, uc * 512:(uc + 1) * 512],
                             start=True, stop=True)
            evac(uc, st[:, uc * 512:(uc + 1) * 512], pr[:])
        nc.scalar.dma_start(pout[tb * 128:(tb + 1) * 128, :], st[:])


def build_program():
    nc = bacc.Bacc("TRN2", target_bir_lowering=False, debug=False)
    io = (
        nc.dram_tensor("xTin", [D, T], F16, kind="ExternalInput").ap(),
        nc.dram_tensor("xeT", [128, 8 * 2 * NE], F16, kind="ExternalInput").ap(),
        nc.dram_tensor("wqT", [128, D], F16, kind="ExternalInput").ap(),
        nc.dram_tensor("wkT", [128, D], F16, kind="ExternalInput").ap(),
        nc.dram_tensor("wvT", [128, D], F16, kind="ExternalInput").ap(),
        nc.dram_tensor("woT", [128, D], F16, kind="ExternalInput").ap(),
        nc.dram_tensor("cosT", [128, T], F16, kind="ExternalInput").ap(),
        nc.dram_tensor("sinT", [128, T], F16, kind="ExternalInput").ap(),
        nc.dram_tensor("ceT", [128, 2 * NE], F16, kind="ExternalInput").ap(),
        nc.dram_tensor("seT", [128, 2 * NE], F16, kind="ExternalInput").ap(),
        nc.dram_tensor("cmul", [2 * NG * 128, GW], F16, kind="ExternalInput").ap(),
        nc.dram_tensor("cadd", [2 * NG * 128, GW], F16, kind="ExternalInput").ap(),
        nc.dram_tensor("partial", [T, D], F16, kind="ExternalOutput").ap(),
    )
    from contextlib import ExitStack
    with tile.TileContext(nc) as tc, ExitStack() as ctx:
        _emit(nc, tc, ctx, io)
    nc.compile()
    return nc


# --------------------------------------------------------------------------
# host-side input preparation
# --------------------------------------------------------------------------

def _count_mask():
    """count[h, i, j] (uint8) = multiplicity of dilation-scale coverage."""
    idx = np.arange(T)
    count = np.zeros((H, T, T), np.uint8)
    for X in range(int(math.log2(T))):
        S, r = 2 ** (X + 2), 2 ** X
        same_seg = (idx[:, None] // S) == (idx[None, :] // S)
        for h in range(H):
            ok = (idx % r) == (h % r)
            count[h] += (same_seg & ok[:, None] & ok[None, :]).astype(np.uint8)
    return count


def _extra_rows(h):
    return np.concatenate([h + 128 * np.arange(16), h + 64 + 128 * np.arange(16)])


_STATIC = None


def _static_inputs():
    """Everything that doesn't depend on runtime tensor values."""
    global _STATIC
    if _STATIC is not None:
        return _STATIC
    count = _count_mask()
    # match the reference's fp32 angle arithmetic exactly (t*f rounds in fp32)
    inv_freq = (1.0 / (np.float32(BASE)
                       ** (np.arange(0, D, 2, dtype=np.float32) / np.float32(D)))
                ).astype(np.float32)
    tpos = np.arange(T, dtype=np.float32)

    per_core = []
    ii = np.arange(T)
    for c in range(NCORES):
        heads = [c, c + 8]
        wrows = np.concatenate([np.arange(64 * c, 64 * c + 64),
                                np.arange(512 + 64 * c, 512 + 64 * c + 64)])
        erows = np.concatenate([_extra_rows(h) for h in heads])  # 64
        # RoPE tables in transposed layout [c-dim row, t]
        f = inv_freq[wrows % 512]                                 # [128]
        ang = (tpos[None, :] * f[:, None]).astype(np.float64)     # fp32-rounded angle
        cosT = np.cos(ang).astype(np.float32)
        sign = np.where(np.arange(128) < 64, -1.0, 1.0)[:, None]
        sinT = (np.sin(ang) * sign).astype(np.float32)
        ceT = cosT[:, erows].copy()
        seT = sinT[:, erows].copy()
        # masks [2, NG, 128, GW]
        cmul = np.zeros((2, NG, 128, GW), np.float32)
        cadd = np.full((2, NG, 128, GW), NEG, np.float32)
        for s, h in enumerate(heads):
            ec = _extra_rows(h)
            for b in range(NB):
                rows = slice(128 * b, 128 * b + 128)
                iblk = ii[rows][:, None]
                co = (b % 2) * TW
                jblk = np.arange(128 * b, 128 * b + 128)[None, :]
                cnt = count[h, rows, 128 * b:128 * b + 128].astype(np.float32)
                val = (cnt > 0) & (jblk <= iblk)
                cmul[s, b // 2, :, co:co + 128] = np.where(val, cnt, 0.0)
                cadd[s, b // 2, :, co:co + 128] = np.where(val, 0.0, NEG)
                je = ec[None, :]
                cnt_e = count[h, rows, :][:, ec].astype(np.float32)
                inb = (je >= 128 * b) & (je < 128 * b + 128)
                val_e = (cnt_e > 0) & (je <= iblk) & ~inb
                cmul[s, b // 2, :, co + 128:co + TW] = np.where(val_e, cnt_e, 0.0)
                cadd[s, b // 2, :, co + 128:co + TW] = np.where(val_e, 0.0, NEG)
        per_core.append(dict(
            wrows=wrows, erows=erows,
            cosT=cosT.astype(np.float16), sinT=sinT.astype(np.float16),
            ceT=ceT.astype(np.float16), seT=seT.astype(np.float16),
            cmul=cmul.reshape(2 * NG * 128, GW).astype(np.float16),
            cadd=cadd.reshape(2 * NG * 128, GW).astype(np.float16),
        ))
    _STATIC = per_core
    return per_core


_NC = None


def _program():
    global _NC
    if _NC is None:
        _NC = build_program()
    return _NC


def build_in_maps(x, Wq, Wk, Wv, Wo):
    x2 = np.ascontiguousarray(x.reshape(T, D), np.float32)
    xT2h = np.ascontiguousarray(x2.T).astype(np.float16)
    static = _static_inputs()
    in_maps = []
    for c in range(NCORES):
        st = static[c]
        wr = st["wrows"]
        def dev_wT(w):
            # [128 c, D] slice -> device layout [128 p, (chunk, c)]
            return np.ascontiguousarray(
                w.T.reshape(8, 128, 128).transpose(1, 0, 2).reshape(128, 8 * 128))

        xe = x2[st["erows"], :].T  # [D, 64]
        in_maps.append({
            "xTin": xT2h,
            "xeT": np.ascontiguousarray(
                xe.reshape(8, 128, 2 * NE).transpose(1, 0, 2)
                  .reshape(128, -1)).astype(np.float16),
            "wqT": dev_wT(Wq[wr, :]).astype(np.float16),
            "wkT": dev_wT(Wk[wr, :]).astype(np.float16),
            "wvT": dev_wT(Wv[wr, :]).astype(np.float16),
            "woT": np.ascontiguousarray(Wo[:, wr].T).astype(np.float16),
            "cosT": st["cosT"],
            "sinT": st["sinT"],
            "ceT": st["ceT"],
            "seT": st["seT"],
            "cmul": st["cmul"],
            "cadd": st["cadd"],
        })
    return in_maps


def kernel(x, Wq, bq, Wk, bk, Wv, bv, Wo, bo, **_):
    x = np.asarray(x, np.float32)
    Wq, Wk, Wv, Wo = (np.asarray(a, np.float32) for a in (Wq, Wk, Wv, Wo))
    bq, bk, bv, bo = (np.asarray(a, np.float32) for a in (bq, bk, bv, bo))
    assert not bq.any() and not bk.any(), "nonzero q/k biases unsupported"

    nc = _program()
    in_maps = build_in_maps(x, Wq, Wk, Wv, Wo)
    res = run_bass_kernel_spmd(nc, in_maps, list(range(NCORES)))
    out = np.zeros((T, D), np.float32)
    for r in res.results:
        out += r["partial"].astype(np.float32)
    # v/o biases are exact host-side epilogues: sum_j p_ij = 1 per row
    out += (Wo @ bv + bo)[None, :]
    return out.reshape(1, T, D)
